# revision 8
# baseline (speedup 1.0000x reference)
"""Trainium2 Bass kernel for nn_EpisodicMemory (retrieval_knn).

Strategy (pure data parallel over batch, 8 cores):
  - Host: transpose x -> xT per-core shards (feature-major), fold constants:
      W1c = W1 @ (I - J/256)   (LayerNorm centering folded into W1)
      SK  = [normalize(keys).T | Wg[:256]]            [256, 65]
      VW  = [values@Wv + 1*bv | ones | values@Wg[256:]] [64, 259]
  - Device (per 512-row block, feature-major f32 matmul chain):
      hT = W1c^T xT (+b1c)          -> mean-sq via ones-matmul (J) broadcast
      rs = exp(-0.5 ln(msq + eps))  -> y = hT * rs -> gT = gelu(g_ln*y + b_ln)
      qT = W2^T gT (+b2)            -> ssq via ones-matmul
      rsq = exp(-0.5 ln(ssq))       -> sfn = (SK[:, :64]^T qT) * rsq -> expT = exp(sfn)
      sim row-major = qT^T SK       -> argmax via DVE max8 (col 64 = gate_q)
      bias-pre = expT^T VW          -> cols 0:257 bias, 257 denom, 258 gate_r
      bias = (1/denom) * bias-pre   (tanh == identity for |u|<4e-3, checked on host)
      gate = 0.5 + 0.5*tanh(0.5*(gate_q + gate_r/denom + bg))
      out0 = bias * gate
  - ACT table sets: {natural_log_exp} and {gelu(+tanh)} only, G-batched with a
    lag-G software pipeline so set switches amortize over G blocks.
"""

import sys
import numpy as np

sys.path.insert(0, "/opt/trn_rl_repo")

KEY_DIM = 256
VALUE_DIM = 257
NUM_SLOTS = 64
NCORES = 8
BS = 512  # rows per block
G = 4     # pipeline lag / ACT set batch

_PROGRAM_CACHE = {}


def _build_program(Rc, use_b1, use_b2, use_tanh_bias):
    import concourse.bass as bass
    import concourse.bacc as bacc
    import concourse.tile as tile
    from concourse import mybir

    f32 = mybir.dt.float32
    u32 = mybir.dt.uint32
    AF = mybir.ActivationFunctionType
    ALU = mybir.AluOpType

    NB = Rc // BS
    assert Rc % BS == 0 and NB % G == 0

    nc = bacc.Bacc("TRN2", target_bir_lowering=False, debug=False)

    # ---- DRAM I/O ----
    xT_d = nc.dram_tensor("xT", [KEY_DIM, Rc], f32, kind="ExternalInput").ap()
    wa_d = nc.dram_tensor("W1c", [KEY_DIM, KEY_DIM], f32, kind="ExternalInput").ap()
    wb_d = nc.dram_tensor("W2", [KEY_DIM, KEY_DIM], f32, kind="ExternalInput").ap()
    sk_d = nc.dram_tensor("SK", [KEY_DIM, 65], f32, kind="ExternalInput").ap()
    vw_d = nc.dram_tensor("VW", [NUM_SLOTS, 259], f32, kind="ExternalInput").ap()
    gl_d = nc.dram_tensor("glb", [128, 2], f32, kind="ExternalInput").ap()
    bl_d = nc.dram_tensor("blb", [128, 2], f32, kind="ExternalInput").ap()
    bg_d = nc.dram_tensor("bg05", [128, 1], f32, kind="ExternalInput").ap()
    if use_b1:
        b1_d = nc.dram_tensor("b1c", [1, KEY_DIM], f32, kind="ExternalInput").ap()
    if use_b2:
        b2_d = nc.dram_tensor("b2r", [1, KEY_DIM], f32, kind="ExternalInput").ap()

    out0_d = nc.dram_tensor("out0", [Rc, VALUE_DIM], f32, kind="ExternalOutput").ap()
    gate_d = nc.dram_tensor("gate", [Rc], f32, kind="ExternalOutput").ap()
    idx_d = nc.dram_tensor("tidx", [Rc], u32, kind="ExternalOutput").ap()
    smax_d = nc.dram_tensor("smax", [Rc], f32, kind="ExternalOutput").ap()
    scr_d = nc.dram_tensor("scr", [NB, BS], f32).ap()  # bounce scratch

    with tile.TileContext(nc) as tc:
        import contextlib

        ctx = contextlib.ExitStack()
        with ctx:
            singles = ctx.enter_context(tc.tile_pool(name="singles", bufs=1))
            # constants
            wa = singles.tile([128, 2, 256], f32, tag="wa")
            nc.sync.dma_start(out=wa[:, 0, :], in_=wa_d[0:128, :])
            nc.sync.dma_start(out=wa[:, 1, :], in_=wa_d[128:256, :])
            wb = singles.tile([128, 2, 256], f32, tag="wb")
            nc.sync.dma_start(out=wb[:, 0, :], in_=wb_d[0:128, :])
            nc.sync.dma_start(out=wb[:, 1, :], in_=wb_d[128:256, :])
            sk = singles.tile([128, 2, 65], f32, tag="sk")
            nc.sync.dma_start(out=sk[:, 0, :], in_=sk_d[0:128, :])
            nc.sync.dma_start(out=sk[:, 1, :], in_=sk_d[128:256, :])
            vw = singles.tile([NUM_SLOTS, 259], f32, tag="vw")
            nc.sync.dma_start(out=vw[:], in_=vw_d[:])
            glb = singles.tile([128, 2], f32, tag="glb")
            nc.sync.dma_start(out=glb[:], in_=gl_d[:])
            blb = singles.tile([128, 2], f32, tag="blb")
            nc.sync.dma_start(out=blb[:], in_=bl_d[:])
            bg05 = singles.tile([128, 1], f32, tag="bg05")
            nc.sync.dma_start(out=bg05[:], in_=bg_d[:])
            oC = singles.tile([128, 128], f32, tag="oC")  # 1/256 for mean-sq
            nc.vector.memset(oC[:], 1.0 / 256.0)
            o1 = singles.tile([128, 128], f32, tag="o1")  # ones for sum-sq
            nc.vector.memset(o1[:], 1.0)
            eps5 = singles.tile([128, 1], f32, tag="eps5")
            nc.vector.memset(eps5[:], 1e-5)
            if use_b1:
                b1r = singles.tile([1, 256], f32, tag="b1r")
                nc.sync.dma_start(out=b1r[:], in_=b1_d[:])
            if use_b2:
                b2r = singles.tile([1, 256], f32, tag="b2r")
                nc.sync.dma_start(out=b2r[:], in_=b2_d[:])
            if use_b1 or use_b2:
                onesrow = singles.tile([1, BS], f32, tag="onesrow")
                nc.vector.memset(onesrow[:], 1.0)

            # accumulators (whole core)
            acc_gate = singles.tile([128, NB * 4], f32, tag="acc_gate")
            acc_smax = singles.tile([128, NB * 4], f32, tag="acc_smax")
            acc_idx = singles.tile([128, NB * 4, 8], u32, tag="acc_idx")

            # pools
            pxt = ctx.enter_context(tc.tile_pool(name="pxt", bufs=6))
            phsq = ctx.enter_context(tc.tile_pool(name="phsq", bufs=2))
            pln = ctx.enter_context(tc.tile_pool(name="pln", bufs=3))
            py = ctx.enter_context(tc.tile_pool(name="py", bufs=2))
            pgt = ctx.enter_context(tc.tile_pool(name="pgt", bufs=G + 1))
            pqts = ctx.enter_context(tc.tile_pool(name="pqts", bufs=G + 2))
            pqsq = ctx.enter_context(tc.tile_pool(name="pqsq", bufs=G + 2))
            psfn = ctx.enter_context(tc.tile_pool(name="psfn", bufs=2))
            pexp = ctx.enter_context(tc.tile_pool(name="pexp", bufs=2))
            psims = ctx.enter_context(tc.tile_pool(name="psims", bufs=2))
            pmx = ctx.enter_context(tc.tile_pool(name="pmx", bufs=2))
            psm = ctx.enter_context(tc.tile_pool(name="psm", bufs=4 * G))
            pbt = ctx.enter_context(tc.tile_pool(name="pbt", bufs=2 * G + 2))
            pot = ctx.enter_context(tc.tile_pool(name="pot", bufs=4))
            prm = ctx.enter_context(tc.tile_pool(name="prm", bufs=G + 2))
            # psum pools
            phq = ctx.enter_context(tc.tile_pool(name="phq", bufs=1, space="PSUM"))
            pstat = ctx.enter_context(tc.tile_pool(name="pstat", bufs=2, space="PSUM"))
            psmall = ctx.enter_context(tc.tile_pool(name="psmall", bufs=2, space="PSUM"))

            # per-pair state carried between phases
            state = {}

            def natlog_phase(jL, jQ):
                st = {}  # new state for block jL; jQ results go to state[jQ]
                hh = None
                if jL is not None:
                    xt = pxt.tile([128, 2, BS], f32, tag="xt")
                    nc.sync.dma_start(out=xt[:, 0, :], in_=xT_d[0:128, jL * BS:(jL + 1) * BS])
                    nc.sync.dma_start(out=xt[:, 1, :], in_=xT_d[128:256, jL * BS:(jL + 1) * BS])
                    hh = phq.tile([128, 2, BS], f32, tag="hq")
                    for m in range(2):
                        for k in range(2):
                            nc.tensor.matmul(
                                hh[:, m, :], wa[:, k, m * 128:(m + 1) * 128],
                                xt[:, k, :], start=(k == 0),
                                stop=(k == 1 and not use_b1))
                        if use_b1:
                            nc.tensor.matmul(
                                hh[:, m, :], b1r[:, m * 128:(m + 1) * 128],
                                onesrow[:], start=False, stop=True)
                    hsq = phsq.tile([128, 2, BS], f32, tag="hsq")
                    nc.scalar.activation(hsq[:], hh[:], AF.Square)
                stat = pstat.tile([128, 2, BS], f32, tag="statbp")
                if jL is not None:
                    for k in range(2):
                        nc.tensor.matmul(stat[:, 0, :], oC[:], hsq[:, k, :],
                                         start=(k == 0), stop=(k == 1))
                else:
                    nc.vector.memset(stat[:, 0, :], 1.0)
                if jQ is not None:
                    qsq = state[jQ]["qsq"]
                    for k in range(2):
                        nc.tensor.matmul(stat[:, 1, :], o1[:], qsq[:, k, :],
                                         start=(k == 0), stop=(k == 1))
                else:
                    nc.vector.memset(stat[:, 1, :], 1.0)
                lt = pln.tile([128, 2, BS], f32, tag="lt")
                nc.scalar.activation(lt[:], stat[:], AF.Ln, bias=eps5[:], scale=1.0)
                rsx = pln.tile([128, 2, BS], f32, tag="rsx")
                nc.scalar.activation(rsx[:], lt[:], AF.Exp, scale=-0.5)
                st["rsx"] = rsx
                if jL is not None:
                    yt = py.tile([128, 2, BS], f32, tag="yt")
                    for m in range(2):
                        nc.vector.tensor_mul(yt[:, m, :], hh[:, m, :], rsx[:, 0, :])
                    st["yt"] = yt
                    st["xt"] = xt
                if jQ is not None:
                    stq = state[jQ]
                    qts = stq["qts"]
                    # bounce rsq strip -> row-major [128, 4]
                    nc.sync.dma_start(out=scr_d[jQ, :], in_=rsx[0:1, 1, :])
                    rqm = prm.tile([128, 4], f32, tag="rqm")
                    nc.sync.dma_start(
                        out=rqm[:], in_=scr_d[jQ, :].rearrange("(c p) -> p c", p=128))
                    # sf = SK[:, :64]^T q  (feature-major sim)
                    sf = psmall.tile([NUM_SLOTS, BS], f32, tag="small")
                    for k in range(2):
                        nc.tensor.matmul(sf[:], sk[:, k, 0:64], qts[:, k, :],
                                         start=(k == 0), stop=(k == 1))
                    sfn = psfn.tile([NUM_SLOTS, BS], f32, tag="sfn")
                    nc.vector.tensor_mul(sfn[:], sf[:], rsx[0:64, 1, :])
                    expT = pexp.tile([NUM_SLOTS, BS], f32, tag="expT")
                    nc.scalar.activation(expT[:], sfn[:], AF.Exp)
                    # row-major sim (+ gate_q col): [128, 4, 65] in one psum bank
                    sim = psmall.tile([128, 4, 65], f32, tag="small")
                    for s in range(4):
                        for k in range(2):
                            nc.tensor.matmul(
                                sim[:, s, :], qts[:, k, s * 128:(s + 1) * 128],
                                sk[:, k, :], start=(k == 0), stop=(k == 1))
                    # bias matmuls, two bank-aligned halves
                    rd = psm.tile([128, 4], f32, tag="rd")
                    tg = psm.tile([128, 4], f32, tag="tg")
                    bts = []
                    for h in range(2):
                        bp = pstat.tile([128, 2, BS], f32, tag="statbp")
                        for s2 in range(2):
                            s = h * 2 + s2
                            nc.tensor.matmul(bp[:, s2, 0:259],
                                             expT[:, s * 128:(s + 1) * 128],
                                             vw[:], start=True, stop=True)
                        nc.vector.reciprocal(rd[:, h * 2:h * 2 + 2],
                                             bp[:, :, 257])
                        nc.vector.tensor_mul(tg[:, h * 2:h * 2 + 2],
                                             bp[:, :, 258], rd[:, h * 2:h * 2 + 2])
                        bt = pbt.tile([128, 2, VALUE_DIM], f32, tag="bt")
                        for s2 in range(2):
                            nc.vector.tensor_scalar_mul(
                                bt[:, s2, :], bp[:, s2, 0:257],
                                rd[:, h * 2 + s2:h * 2 + s2 + 1])
                        bts.append(bt)
                    # gate logit: tg = gate_r/denom + gate_q
                    nc.vector.tensor_add(tg[:], tg[:], sim[:, :, 64])
                    stq["tg"] = tg
                    stq["rd"] = rd
                    stq["bts"] = bts
                    # argmax over slots (row-major)
                    sims = psims.tile([128, 4, 64], f32, tag="sims")
                    nc.vector.tensor_copy(sims[:], sim[:, :, 0:64])
                    mx = pmx.tile([128, 4, 8], f32, tag="mx")
                    for s in range(4):
                        nc.vector.max(out=mx[:, s, :], in_=sims[:, s, :])
                        nc.vector.max_index(out=acc_idx[:, jQ * 4 + s, :],
                                            in_max=mx[:, s, :], in_values=sims[:, s, :])
                    nc.vector.tensor_mul(acc_smax[:, jQ * 4:(jQ + 1) * 4],
                                         mx[:, :, 0], rqm[:])
                if jL is not None:
                    state[jL] = st
                return st

            def gelu_phase(jL, jQ, st):
                if jQ is not None:
                    stq = state[jQ]
                    th = psm.tile([128, 4], f32, tag="th")
                    nc.scalar.activation(th[:], stq["tg"], AF.Tanh,
                                         bias=bg05[:], scale=0.5)
                    stq["th"] = th
                    if use_tanh_bias:
                        for bt in stq["bts"]:
                            nc.scalar.activation(bt[:], bt[:], AF.Tanh)
                if jL is not None:
                    yt = st["yt"]
                    gt = pgt.tile([128, 2, BS], f32, tag="gt")
                    for m in range(2):
                        nc.scalar.activation(gt[:, m, :], yt[:, m, :], AF.Gelu,
                                             bias=blb[:, m:m + 1], scale=glb[:, m:m + 1])
                    st["gt"] = gt

            def tail_phase(jL, jQ, st):
                if jQ is not None:
                    stq = state[jQ]
                    nc.vector.tensor_scalar(
                        acc_gate[:, jQ * 4:(jQ + 1) * 4], stq["th"], 0.5, 0.5,
                        op0=ALU.mult, op1=ALU.add)
                    for h in range(2):
                        bt = stq["bts"][h]
                        ot = pot.tile([128, 2, VALUE_DIM], f32, tag="ot")
                        for s2 in range(2):
                            s = h * 2 + s2
                            nc.gpsimd.tensor_scalar_mul(
                                ot[:, s2, :], bt[:, s2, :],
                                acc_gate[:, jQ * 4 + s:jQ * 4 + s + 1])
                            nc.sync.dma_start(
                                out=out0_d[jQ * BS + s * 128:jQ * BS + (s + 1) * 128, :],
                                in_=ot[:, s2, :])
                if jL is not None:
                    gt = st["gt"]
                    qq = phq.tile([128, 2, BS], f32, tag="hq")
                    for m in range(2):
                        for k in range(2):
                            nc.tensor.matmul(
                                qq[:, m, :], wb[:, k, m * 128:(m + 1) * 128],
                                gt[:, k, :], start=(k == 0),
                                stop=(k == 1 and not use_b2))
                        if use_b2:
                            nc.tensor.matmul(
                                qq[:, m, :], b2r[:, m * 128:(m + 1) * 128],
                                onesrow[:], start=False, stop=True)
                    qts = pqts.tile([128, 2, BS], f32, tag="qts")
                    nc.vector.tensor_copy(qts[:, 0, :], qq[:, 0, :])
                    nc.scalar.copy(qts[:, 1, :], qq[:, 1, :])
                    qsq = pqsq.tile([128, 2, BS], f32, tag="qsq")
                    nc.gpsimd.tensor_mul(qsq[:], qts[:], qts[:])
                    st["qts"] = qts
                    st["qsq"] = qsq

            for jj in range(0, NB + G, G):
                sts = []
                for t in range(G):
                    jL = jj + t if jj + t < NB else None
                    jQ = jj + t - G if jj + t - G >= 0 else None
                    if jL is None and jQ is None:
                        sts.append(None)
                        continue
                    sts.append((jL, jQ, natlog_phase(jL, jQ)))
                for item in sts:
                    if item is not None:
                        gelu_phase(*item)
                for item in sts:
                    if item is not None:
                        tail_phase(*item)
                # free consumed per-block state
                for item in sts:
                    if item is not None and item[1] is not None:
                        state.pop(item[1], None)

            # final accumulator write-out
            nc.sync.dma_start(
                out=gate_d.rearrange("(p c) -> p c", p=128), in_=acc_gate[:])
            nc.sync.dma_start(
                out=smax_d.rearrange("(p c) -> p c", p=128), in_=acc_smax[:])
            nc.sync.dma_start(
                out=idx_d.rearrange("(p c) -> p c", p=128), in_=acc_idx[:, :, 0])

    nc.compile()
    return nc


def _get_program(Rc, use_b1, use_b2, use_tanh_bias):
    key = (Rc, use_b1, use_b2, use_tanh_bias)
    if key not in _PROGRAM_CACHE:
        _PROGRAM_CACHE[key] = _build_program(*key)
    return _PROGRAM_CACHE[key]


def kernel(x, keys, values, W1, b1, g_ln, b_ln, W2, b2, Wv, bv, Wg, bg):
    from concourse.bass_utils import run_bass_kernel_spmd

    x = np.ascontiguousarray(np.asarray(x, np.float32))
    B = x.shape[0]
    Rc = B // NCORES
    f64 = np.float64
    W1_, W2_, Wv_, Wg_ = (np.asarray(a, f64) for a in (W1, W2, Wv, Wg))
    keys_, values_ = np.asarray(keys, f64), np.asarray(values, f64)
    b1_, b2_, bv_, bg_ = (np.asarray(a, f64) for a in (b1, b2, bv, bg))

    P = np.eye(KEY_DIM) - np.ones((KEY_DIM, KEY_DIM)) / KEY_DIM
    W1c = np.ascontiguousarray((W1_ @ P).astype(np.float32))
    b1c = (b1_ @ P).astype(np.float32)
    kn = keys_ / np.sqrt((keys_ ** 2).sum(-1, keepdims=True) + 1e-12)
    SK = np.ascontiguousarray(
        np.concatenate([kn.T, Wg_[:KEY_DIM, 0:1]], axis=1).astype(np.float32))
    VWv = values_ @ Wv_ + bv_[None, :]
    vg = values_ @ Wg_[KEY_DIM:, 0]
    VW = np.ascontiguousarray(
        np.concatenate([VWv, np.ones((NUM_SLOTS, 1)), vg[:, None]],
                       axis=1).astype(np.float32))
    glb = np.ascontiguousarray(
        np.asarray(g_ln, np.float32).reshape(2, 128).T.copy())
    blb = np.ascontiguousarray(
        np.asarray(b_ln, np.float32).reshape(2, 128).T.copy())
    bg05 = np.full((128, 1), 0.5 * float(bg_.reshape(-1)[0]), np.float32)

    use_b1 = bool(np.any(b1c != 0))
    use_b2 = bool(np.any(np.asarray(b2) != 0))
    # tanh(u) ~= u when |u| <= max_s |VWv[s, v]| is tiny (attn is convex comb.)
    use_tanh_bias = bool(np.abs(VWv).max() > 4e-3)

    nc = _get_program(Rc, use_b1, use_b2, use_tanh_bias)

    xT = np.ascontiguousarray(x.T)  # [256, B]
    in_maps = []
    for c in range(NCORES):
        m = {
            "xT": np.ascontiguousarray(xT[:, c * Rc:(c + 1) * Rc]),
            "W1c": W1c, "W2": np.ascontiguousarray(W2_.astype(np.float32)),
            "SK": SK, "VW": VW, "glb": glb, "blb": blb, "bg05": bg05,
        }
        if use_b1:
            m["b1c"] = np.ascontiguousarray(b1c.reshape(1, KEY_DIM))
        if use_b2:
            m["b2r"] = np.ascontiguousarray(
                np.asarray(b2, np.float32).reshape(1, KEY_DIM))
        in_maps.append(m)

    res = run_bass_kernel_spmd(nc, in_maps, list(range(NCORES))).results

    NB = Rc // BS
    out0 = np.empty((B, VALUE_DIM), np.float32)
    gate = np.empty((B,), np.float32)
    tidx = np.empty((B,), np.int32)
    smax = np.empty((B,), np.float32)

    def unshuf(a):  # device order (p, blk, s) -> row 512*blk + 128*s + p
        return np.ascontiguousarray(
            a.reshape(128, NB, 4).transpose(1, 2, 0).reshape(-1))

    for c in range(NCORES):
        r = res[c]
        sl = slice(c * Rc, (c + 1) * Rc)
        out0[sl] = r["out0"]
        gate[sl] = unshuf(r["gate"])
        tidx[sl] = unshuf(r["tidx"]).astype(np.int32)
        smax[sl] = unshuf(r["smax"])
    return out0, gate.reshape(B, 1), tidx, smax


if __name__ == "__main__":
    import reference as R
    inp = {k: np.asarray(v) for k, v in R.setup_inputs().items()}
    outs = kernel(**inp)
    print([o.shape for o in outs])
